# revision 1
# baseline (speedup 1.0000x reference)
"""Self-contained Trainium2 (Bass/Tile) kernel for the SNN problem.

kernel(**inputs) takes FULL unsharded inputs (as from setup_inputs()) and
returns the FULL [64, 2] float32 output. Internally: batch-sharded SPMD over
8 NeuronCores; BN statistics exchanged via 5 tiny AllReduces; LIF scans
chunk-parallel (512 chunks x 16 steps, 12 warm); LSNN chunk-parallel
(128 chunks x 64 steps, 96 warm); LI readout via exact linear closed form.
"""
import json
import numpy as np
from contextlib import ExitStack
import bass_rust
from concourse import bass, mybir, tile, bass2jax
from concourse.bass_utils import run_bass_kernel_spmd


# ---------- walrus workarounds (this container's compiler build) ----------
def _no_clear(self, sems):
    if not sems:
        return
    sem_nums = [s.num if hasattr(s, "num") else s for s in sems]
    self._state.prepend_free_semaphores(sem_nums)
    for ps in self._tile_sem_poison_stack:
        ps.update(sem_nums)


def _split_multi_waits(bir_json):
    d = json.loads(bir_json)
    changed = False
    for fn in d.get("functions", []):
        for blk in fn.get("blocks", []):
            insts = blk.get("instructions", [])
            out = []
            for inst in insts:
                si = inst.get("sync_info")
                if si:
                    ow = si.get("on_wait") or []
                    if len(ow) > 1:
                        changed = True
                        for j, w in enumerate(ow[:-1]):
                            out.append({
                                "debug": inst.get("debug", 0),
                                "engine": inst["engine"],
                                "ins": [], "outs": [],
                                "name": f"{inst['name']}-wsplit{j}",
                                "opcode": "NoOp",
                                "sync_info": {"on_update": [], "on_wait": [w]},
                            })
                        si["on_wait"] = [ow[-1]]
                out.append(inst)
            if len(out) != len(insts):
                blk["instructions"] = out
    return json.dumps(d).encode() if changed else bir_json


_orig_compile = bass2jax.compile_bir_kernel


def _patched_compile(bir_json, tmpdir, neff_name="file.neff"):
    return _orig_compile(_split_multi_waits(bir_json), tmpdir, neff_name=neff_name)


def _apply_patches():
    bass.Bass.clear_and_free_semaphores = _no_clear
    bass2jax.compile_bir_kernel = _patched_compile


_apply_patches()

DT = mybir.dt
F32 = DT.float32
BF16 = DT.bfloat16
OP = mybir.AluOpType
AFT = mybir.ActivationFunctionType

N_CORES = 8
BL, C, T, H, O = 8, 64, 8192, 10, 2
P = BL * H  # 80
VTH = 0.5
TAU = 0.25
NCH, LC, WC = 512, 16, 12          # LIF chunks / len / warm
STEPS = LC + WC                     # 28
NCH2, LC2, WC2 = 128, 64, 96        # LSNN
STEPS2 = LC2 + WC2                  # 160
PADY = WC + T                       # 8204
HALO = 4
SPW = 2 * HALO + T                  # 8200
BIGW = WC2 + T                      # 8288 (bigA width)
EPS = 1e-5
NTOT = 64 * T




def _finish_early(nc, out, pst, tap_handles, ctx):
    t = pst.tile([16, 1], bass.mybir.dt.float32, tag="outt")
    nc.vector.memset(t[:], 0.0)
    nc.sync.dma_start(out=out[:], in_=t[:])
    ctx.close()
    return nc, tap_handles

def build_kernel(taps=(), stage_limit=99, repeat=1):
    nc = bass.Bass()
    taps = set(taps)

    eeg = nc.declare_dram_parameter("eeg", [BL, C, T], F32, isOutput=False)
    wf2 = nc.declare_dram_parameter("wf2", [128, H], F32, isOutput=False)
    prm = {}
    for nm, shape, dt in [
            ("bd_q", [P, P], F32), ("bd_k", [P, P], F32), ("bd_v", [P, P], F32),
            ("bd_c", [P, P], F32), ("bd_in", [P, P], F32), ("bd_rec", [P, P], F32),
            ("s_sel", [P, P], BF16), ("i80f", [P, P], F32),
            ("ssum", [P, H], F32), ("r10", [H, P], F32),
            ("wt80", [P, T], F32), ("mcls", [P, 16], F32), ("gb", [H, 10], F32)]:
        prm[nm] = nc.declare_dram_parameter(nm, shape, dt, isOutput=False)

    out = nc.declare_dram_parameter("out", [16, 1], F32, isOutput=True)

    tap_handles = {}

    def tapdecl(name, shape, dtype):
        if name in taps:
            tap_handles[name] = nc.declare_dram_parameter(
                f"tap_{name}", shape, dtype, isOutput=True)
        return tap_handles.get(name)

    t_yf = tapdecl("yf", [P, PADY], F32)
    t_xspk = tapdecl("xspk", [P, T], F32)
    t_q = tapdecl("qspk", [P, SPW], BF16)
    t_k = tapdecl("kspk", [P, SPW], BF16)
    t_v = tapdecl("vspk", [P, SPW], F32)
    t_avx = tapdecl("avx", [P, BIGW], F32)
    t_avspk = tapdecl("avspk", [P, T], F32)
    t_ycma = tapdecl("ycma", [P, PADY], F32)
    t_av2 = tapdecl("av2spk", [P, T], F32)
    t_xw = tapdecl("xw", [P, BIGW], F32)
    t_zs = tapdecl("zs", [P, T], F32)
    t_stats = tapdecl("stats", [P, 16], F32)
    t_inner = tapdecl("inner", [P, 1], F32)

    ctx = ExitStack()
    with tile.TileContext(nc) as tc:
        pp = ctx.enter_context(tc.tile_pool(name="params", bufs=1))
        pbig = ctx.enter_context(tc.tile_pool(name="big", bufs=1))
        pio = ctx.enter_context(tc.tile_pool(name="io", bufs=2))
        psc = ctx.enter_context(tc.tile_pool(name="scan", bufs=2))
        psc1 = ctx.enter_context(tc.tile_pool(name="scan1", bufs=1))
        pst = ctx.enter_context(tc.tile_pool(name="stats", bufs=1))
        pps1 = ctx.enter_context(tc.tile_pool(name="psum1", bufs=1, space="PSUM"))
        pps2 = ctx.enter_context(tc.tile_pool(name="psum2", bufs=2, space="PSUM"))
        pdram = ctx.enter_context(tc.tile_pool(name="dram", bufs=1, space="DRAM"))

        def ldparam(name):
            src = prm[name]
            t = pp.tile(list(src.shape), src.dtype, tag=f"prm_{name}")
            nc.sync.dma_start(out=t[:], in_=src[:])
            return t

        wf2s = pp.tile([128, H], F32, tag="prm_wf2")
        nc.sync.dma_start(out=wf2s[:], in_=wf2[:])
        bdq, bdk, bdv = ldparam("bd_q"), ldparam("bd_k"), ldparam("bd_v")
        bdc, bdin, bdrec = ldparam("bd_c"), ldparam("bd_in"), ldparam("bd_rec")
        ssel, identf = ldparam("s_sel"), ldparam("i80f")
        ssums, r10s = ldparam("ssum"), ldparam("r10")
        mclss, gbs = ldparam("mcls"), ldparam("gb")

        bigA = pbig.tile([P, BIGW], F32, tag="A")
        spkQ = pbig.tile([P, SPW], BF16, tag="SQ")
        spkK = pbig.tile([P, SPW], BF16, tag="SK")
        spkV = pbig.tile([P, SPW], F32, tag="SV")
        coefs = pst.tile([P, 16], F32, tag="coefs")

        # ---------------- BN stats -> global affine coefs ----------------
        def bn_to_global(ybufs, gcols, coffs, ar_name):
            nl = len(ybufs)
            K = 2 * nl
            strip = pst.tile([P, nl, 16, 6], F32, tag="bnstrip")
            for li, y in enumerate(ybufs):
                for j in range(16):
                    nc.vector.bn_stats(out=strip[:, li, j, :],
                                       in_=y[:, WC + j * 512: WC + (j + 1) * 512])
            ss = pst.tile([P, K], F32, tag="ss")
            mv = pst.tile([P, nl, 2], F32, tag="mv")
            for li in range(nl):
                nc.vector.bn_aggr(out=mv[:, li, :], in_=strip[:, li, :, :])
                mean = mv[:, li, 0:1]
                var = mv[:, li, 1:2]
                nc.vector.tensor_scalar(out=ss[:, 2 * li:2 * li + 1], in0=mean,
                                        scalar1=float(T), scalar2=None, op0=OP.mult)
                sq = ss[:, 2 * li + 1:2 * li + 2]
                nc.vector.scalar_tensor_tensor(out=sq, in0=mean, scalar=1.0,
                                               in1=mean, op0=OP.mult, op1=OP.mult)
                nc.vector.scalar_tensor_tensor(out=sq, in0=sq, scalar=1.0,
                                               in1=var, op0=OP.mult, op1=OP.add)
                nc.vector.tensor_scalar(out=sq, in0=sq, scalar1=float(T),
                                        scalar2=None, op0=OP.mult)
            pred = pps2.tile([H, K], F32, tag="small")
            nc.tensor.matmul(out=pred[:], lhsT=ssums[:], rhs=ss[:],
                             start=True, stop=True)
            loc = pst.tile([H, K], F32, tag="loc")
            nc.vector.tensor_copy(out=loc[:], in_=pred[:])
            ar_in = pdram.tile([H, K], F32, tag="arin")
            ar_out = pdram.tile([H, K], F32, tag="arout",
                                addr_space="Shared")
            nc.gpsimd.dma_start(out=ar_in[:], in_=loc[:])
            nc.gpsimd.collective_compute(
                "AllReduce", OP.add, replica_groups=[list(range(N_CORES))],
                ins=[ar_in.opt()], outs=[ar_out.opt()])
            glob = pst.tile([H, K], F32, tag="glob")
            nc.gpsimd.dma_start(out=glob[:], in_=ar_out[:])
            ac10 = pst.tile([H, K], F32, tag="ac10")
            tmp = pst.tile([H, 4], F32, tag="tmp")
            for li, gcol in enumerate(gcols):
                mean, var = tmp[:, 0:1], tmp[:, 1:2]
                nc.vector.tensor_scalar(out=mean, in0=glob[:, 2 * li:2 * li + 1],
                                        scalar1=1.0 / NTOT, scalar2=None, op0=OP.mult)
                nc.vector.tensor_scalar(out=var, in0=glob[:, 2 * li + 1:2 * li + 2],
                                        scalar1=1.0 / NTOT, scalar2=None, op0=OP.mult)
                nc.vector.scalar_tensor_tensor(out=tmp[:, 2:3], in0=mean, scalar=1.0,
                                               in1=mean, op0=OP.mult, op1=OP.mult)
                nc.vector.tensor_tensor(out=var, in0=var, in1=tmp[:, 2:3],
                                        op=OP.subtract)
                nc.vector.tensor_scalar(out=var, in0=var, scalar1=EPS,
                                        scalar2=None, op0=OP.add)
                nc.scalar.activation(out=var, in_=var, func=AFT.Sqrt)
                nc.vector.reciprocal(out=var, in_=var)
                a10 = ac10[:, 2 * li:2 * li + 1]
                nc.vector.scalar_tensor_tensor(out=a10, in0=var, scalar=VTH,
                                               in1=gbs[:, gcol:gcol + 1],
                                               op0=OP.mult, op1=OP.mult)
                nc.vector.scalar_tensor_tensor(out=tmp[:, 3:4], in0=a10, scalar=-1.0,
                                               in1=mean, op0=OP.mult, op1=OP.mult)
                nc.vector.tensor_tensor(out=ac10[:, 2 * li + 1:2 * li + 2],
                                        in0=gbs[:, gcol + 1:gcol + 2],
                                        in1=tmp[:, 3:4], op=OP.add)
            pbc = pps2.tile([P, K], F32, tag="small")
            nc.tensor.matmul(out=pbc[:], lhsT=r10s[:], rhs=ac10[:],
                             start=True, stop=True)
            for li, coff in enumerate(coffs):
                nc.vector.tensor_copy(out=coefs[:, coff:coff + 2],
                                      in_=pbc[:, 2 * li:2 * li + 2])

        def affine_inplace(ybuf, coff):
            a = coefs[:, coff:coff + 1]
            cst = coefs[:, coff + 1:coff + 2]
            for j in range(8):
                sl = ybuf[:, WC + j * 1024: WC + (j + 1) * 1024]
                nc.scalar.activation(out=sl, in_=sl, func=AFT.Identity,
                                     bias=cst, scale=a)

        # ---------------- chunk-parallel LIF scan ----------------
        def lif_scan(xcol_fn, spk_fn, theta, tag, spike_val=None, extra_theta_col=False):
            """spk buffer gets (scr>=theta)*spike_val (default theta-valued).
            reset: u_keep = min(scr, theta) - theta*spike  (bitwise == u*(1-s)).
            If spike_val != theta, a separate theta-valued scratch is kept."""
            sval = theta if spike_val is None else spike_val
            shp = [P, NCH]
            u_prev = psc.tile(shp, F32, tag=f"{tag}_u")
            nc.vector.memset(u_prev[:], 0.0)
            for s in range(STEPS):
                scr = psc1.tile(shp, F32, tag=f"{tag}_s")
                nc.vector.scalar_tensor_tensor(out=scr[:], in0=u_prev[:],
                                               scalar=TAU, in1=xcol_fn(s),
                                               op0=OP.mult, op1=OP.add)
                minp = psc1.tile(shp, F32, tag=f"{tag}_m")
                nc.vector.tensor_scalar(out=minp[:], in0=scr[:], scalar1=theta,
                                        scalar2=None, op0=OP.min)
                if s >= WC:
                    spk_col = spk_fn(s - WC)
                    nc.vector.tensor_scalar(out=spk_col, in0=scr[:], scalar1=theta,
                                            scalar2=sval, op0=OP.is_ge, op1=OP.mult)
                    if extra_theta_col:
                        thcol = psc1.tile(shp, F32, tag=f"{tag}_th")
                        nc.vector.tensor_scalar(out=thcol[:], in0=scr[:], scalar1=theta,
                                                scalar2=theta, op0=OP.is_ge, op1=OP.mult)
                        sub_src = thcol[:]
                    else:
                        sub_src = spk_col
                else:
                    thcol = psc1.tile(shp, F32, tag=f"{tag}_th")
                    nc.vector.tensor_scalar(out=thcol[:], in0=scr[:], scalar1=theta,
                                            scalar2=theta, op0=OP.is_ge, op1=OP.mult)
                    sub_src = thcol[:]
                u_new = psc.tile(shp, F32, tag=f"{tag}_u")
                nc.gpsimd.tensor_tensor(out=u_new[:], in0=minp[:], in1=sub_src,
                                        op=OP.subtract)
                u_prev = u_new

        for _rep in range(repeat):
            # ================= S1: front matmul =================
            yf_t = pbig.tile([P, PADY], F32, tag="Y")
            yf = yf_t[:]
            nc.vector.memset(yf[:, 0:WC], 0.0)
            for pair in range(4):
                b0, b1 = 2 * pair, 2 * pair + 1
                for quar in range(4):
                    ps = pps1.tile([42, 2048], F32, tag="big4")
                    for i in range(2):
                        tl = pio.tile([128, 1024], F32, tag="eegtile")
                        base = quar * 2048 + i * 1024
                        nc.sync.dma_start(out=tl[0:64, :], in_=eeg[b0][:, base:base + 1024])
                        nc.sync.dma_start(out=tl[64:128, :], in_=eeg[b1][:, base:base + 1024])
                        for c5 in range(2):
                            col = i * 1024 + c5 * 512
                            nc.tensor.matmul(out=ps[0:H, col:col + 512],
                                             lhsT=wf2s[0:64, :],
                                             rhs=tl[0:64, c5 * 512:(c5 + 1) * 512],
                                             start=True, stop=True)
                            nc.tensor.matmul(out=ps[32:32 + H, col:col + 512],
                                             lhsT=wf2s[64:128, :],
                                             rhs=tl[64:128, c5 * 512:(c5 + 1) * 512],
                                             start=True, stop=True,
                                             tile_position=(64, 32))
                    for hh in range(2):
                        stage = pio.tile([42, 1024], F32, tag="rr")
                        nc.scalar.activation(out=stage[:], in_=ps[:, hh * 1024:(hh + 1) * 1024],
                                             func=AFT.Copy)
                        c0 = WC + quar * 2048 + hh * 1024
                        nc.sync.dma_start(out=yf[b0 * H:(b0 + 1) * H, c0:c0 + 1024],
                                          in_=stage[0:H, :])
                        nc.sync.dma_start(out=yf[b1 * H:(b1 + 1) * H, c0:c0 + 1024],
                                          in_=stage[32:32 + H, :])

            if stage_limit < 2:
                return _finish_early(nc, out, pst, tap_handles, ctx)
            # ================= S2: front BN =================
            bn_to_global([yf], [0], [0], "f")
            affine_inplace(yf, 0)
            if t_yf is not None:
                nc.sync.dma_start(out=t_yf[:], in_=yf)

            if stage_limit < 3:
                return _finish_early(nc, out, pst, tap_handles, ctx)
            # ================= S3: front LIF -> x spikes =================
            xspk = bigA[:, 0:T]
            lif_scan(lambda s: yf[:, s: s + (NCH - 1) * LC + 1: LC],
                     lambda j: xspk[:, j: j + (NCH - 1) * LC + 1: LC],
                     VTH, "sc")
            if t_xspk is not None:
                nc.sync.dma_start(out=t_xspk[:], in_=xspk)

            if stage_limit < 4:
                return _finish_early(nc, out, pst, tap_handles, ctx)
            # ================= S4/S5: q,k,v per-layer matmul+BN+LIF =================
            for buf in (spkQ, spkK, spkV):
                nc.vector.memset(buf[:, 0:HALO], 0.0)
                nc.vector.memset(buf[:, HALO + T:], 0.0)
            V_VAL = float(np.float32(0.1) * np.float32(4.0))  # exact 4*fl(0.1)
            qkv_dst = {"q": (spkQ, None), "k": (spkK, None), "v": (spkV, V_VAL)}
            for li, (name, bd) in enumerate((("q", bdq), ("k", bdk), ("v", bdv))):
                ysl_t = pbig.tile([P, PADY], F32, tag="Y")
                ysl = ysl_t[:]
                nc.vector.memset(ysl[:, 0:WC], 0.0)
                for j in range(16):
                    ps = pps2.tile([P, 512], F32, tag="small")
                    nc.tensor.matmul(out=ps[:], lhsT=bd[:],
                                     rhs=xspk[:, j * 512:(j + 1) * 512],
                                     start=True, stop=True)
                    nc.scalar.activation(out=ysl[:, WC + j * 512: WC + (j + 1) * 512],
                                         in_=ps[:], func=AFT.Copy)
                bn_to_global([ysl], [2 + 2 * li], [2 + 2 * li], name)
                affine_inplace(ysl, 2 + 2 * li)
                sbuf, sval = qkv_dst[name]
                lif_scan(lambda s, y=ysl: y[:, s: s + (NCH - 1) * LC + 1: LC],
                         lambda j, sb=sbuf: sb[:, HALO + j: HALO + j + (NCH - 1) * LC + 1: LC],
                         VTH, "sc", spike_val=sval, extra_theta_col=(sval is not None))
            if t_q is not None:
                nc.sync.dma_start(out=t_q[:], in_=spkQ[:])
                nc.sync.dma_start(out=t_k[:], in_=spkK[:])
                nc.sync.dma_start(out=t_v[:], in_=spkV[:])

            if stage_limit < 6:
                return _finish_early(nc, out, pst, tap_handles, ctx)
            # ================= S6: banded attention =================
            avx = bigA  # av values at [WC2:], zero head
            nc.vector.memset(avx[:, 0:WC2], 0.0)
            for blk in range(8):
                psav = pps1.tile([P, 1024], F32, tag="med")
                pcnt2 = pps1.tile([P, 2048], F32, tag="big4")
                for d in range(9):
                    prod = pio.tile([P, 1024], BF16, tag="prod")
                    q_ap = spkQ[:, HALO + blk * 1024: HALO + (blk + 1) * 1024]
                    k_ap = spkK[:, d + blk * 1024: d + blk * 1024 + 1024]
                    v_ap = spkV[:, d + blk * 1024: d + blk * 1024 + 1024]
                    nc.vector.tensor_tensor(out=prod[:], in0=q_ap, in1=k_ap, op=OP.mult)
                    pcnt = pcnt2[:, (d % 2) * 1024: (d % 2 + 1) * 1024]
                    for c2 in range(2):
                        nc.tensor.matmul(out=pcnt[:, c2 * 512:(c2 + 1) * 512],
                                         lhsT=ssel[:],
                                         rhs=prod[:, c2 * 512:(c2 + 1) * 512],
                                         start=True, stop=True)
                    rr = pio.tile([P, 1024], F32, tag="rr")
                    nc.vector.tensor_tensor(out=rr[:], in0=pcnt[:], in1=v_ap, op=OP.mult)
                    for c2 in range(2):
                        nc.tensor.matmul(out=psav[:, c2 * 512:(c2 + 1) * 512],
                                         lhsT=identf[:],
                                         rhs=rr[:, c2 * 512:(c2 + 1) * 512],
                                         start=(d == 0), stop=(d == 8))
                nc.scalar.activation(out=avx[:, WC2 + blk * 1024: WC2 + (blk + 1) * 1024],
                                     in_=psav[:], func=AFT.Copy)
            if t_avx is not None:
                nc.sync.dma_start(out=t_avx[:], in_=avx[:])

            if stage_limit < 7:
                return _finish_early(nc, out, pst, tap_handles, ctx)
            # ================= S7: av LIF (scaled by 10: theta=5) =================
            avspk_t = pbig.tile([P, T], F32, tag="SA")
            avspk = avspk_t[:]
            lif_scan(lambda s: avx[:, (WC2 - WC) + s: (WC2 - WC) + s + (NCH - 1) * LC + 1: LC],
                     lambda j: avspk[:, j: j + (NCH - 1) * LC + 1: LC],
                     VTH, "sc")
            if t_avspk is not None:
                nc.sync.dma_start(out=t_avspk[:], in_=avspk)

            if stage_limit < 8:
                return _finish_early(nc, out, pst, tap_handles, ctx)
            # ================= S8: cma matmul + BN + LIF =================
            ycma_t = pbig.tile([P, PADY], F32, tag="Y")
            ycma = ycma_t[:]
            nc.vector.memset(ycma[:, 0:WC], 0.0)
            for j in range(16):
                ps = pps2.tile([P, 512], F32, tag="small")
                nc.tensor.matmul(out=ps[:], lhsT=bdc[:],
                                 rhs=avspk[:, j * 512:(j + 1) * 512],
                                 start=True, stop=True)
                nc.scalar.activation(out=ycma[:, WC + j * 512: WC + (j + 1) * 512],
                                     in_=ps[:], func=AFT.Copy)
            bn_to_global([ycma], [8], [8], "c")
            affine_inplace(ycma, 8)
            if t_ycma is not None:
                nc.sync.dma_start(out=t_ycma[:], in_=ycma)
            av2spk_t = pbig.tile([P, T], F32, tag="SA")
            av2spk = av2spk_t[:]
            lif_scan(lambda s: ycma[:, s: s + (NCH - 1) * LC + 1: LC],
                     lambda j: av2spk[:, j: j + (NCH - 1) * LC + 1: LC],
                     VTH, "sc")
            if t_av2 is not None:
                nc.sync.dma_start(out=t_av2[:], in_=av2spk)

            if stage_limit < 10:
                return _finish_early(nc, out, pst, tap_handles, ctx)
            # ================= S10: xw = W_in @ av2spk =================
            xw = bigA  # reuse again (avx dead after av scan)
            nc.vector.memset(xw[:, 0:WC2], 0.0)
            for j in range(16):
                ps = pps2.tile([P, 512], F32, tag="small")
                nc.tensor.matmul(out=ps[:], lhsT=bdin[:],
                                 rhs=av2spk[:, j * 512:(j + 1) * 512],
                                 start=True, stop=True)
                nc.scalar.activation(out=xw[:, WC2 + j * 512: WC2 + (j + 1) * 512],
                                     in_=ps[:], func=AFT.Copy)
            if t_xw is not None:
                nc.sync.dma_start(out=t_xw[:], in_=xw[:])

            if stage_limit < 11:
                return _finish_early(nc, out, pst, tap_handles, ctx)
            # ================= S11: LSNN scan =================
            zs_t = pbig.tile([P, T], F32, tag="SA")
            zs = zs_t[:]
            w_prev = psc.tile([P, NCH2], F32, tag="ls_w")
            i_prev = psc.tile([P, NCH2], F32, tag="ls_i")
            nc.vector.memset(w_prev[:], 0.0)
            nc.vector.memset(i_prev[:], 0.0)
            z_prev = None
            for s in range(STEPS2):
                tscr = psc1.tile([P, NCH2], F32, tag="ls_t")
                nc.vector.scalar_tensor_tensor(out=tscr[:], in0=w_prev[:], scalar=0.9,
                                               in1=i_prev[:], op0=OP.mult, op1=OP.add)
                if s >= WC2:
                    j = s - WC2
                    zdst = zs[:, j: j + (NCH2 - 1) * LC2 + 1: LC2]
                else:
                    ztile = psc.tile([P, NCH2], F32, tag="ls_z")
                    zdst = ztile[:]
                nc.vector.tensor_scalar(out=zdst, in0=tscr[:], scalar1=10.0 * VTH,
                                        scalar2=None, op0=OP.is_ge)
                minp = psc1.tile([P, NCH2], F32, tag="ls_m")
                nc.vector.tensor_scalar(out=minp[:], in0=tscr[:], scalar1=10.0 * VTH,
                                        scalar2=None, op0=OP.min)
                z5 = psc1.tile([P, NCH2], F32, tag="ls_z5")
                nc.gpsimd.tensor_scalar(out=z5[:], in0=zdst, scalar1=10.0 * VTH,
                                        scalar2=None, op0=OP.mult)
                w_new = psc.tile([P, NCH2], F32, tag="ls_w")
                nc.gpsimd.tensor_tensor(out=w_new[:], in0=minp[:], in1=z5[:],
                                        op=OP.subtract)
                psr = pps2.tile([P, NCH2], F32, tag="small")
                nc.tensor.matmul(out=psr[:], lhsT=identf[:],
                                 rhs=xw[:, s: s + (NCH2 - 1) * LC2 + 1: LC2],
                                 start=True, stop=(z_prev is None))
                if z_prev is not None:
                    nc.tensor.matmul(out=psr[:], lhsT=bdrec[:], rhs=z_prev,
                                     start=False, stop=True)
                i_new = psc.tile([P, NCH2], F32, tag="ls_i")
                nc.vector.scalar_tensor_tensor(out=i_new[:], in0=i_prev[:], scalar=0.8,
                                               in1=psr[:], op0=OP.mult, op1=OP.add)
                w_prev, i_prev, z_prev = w_new, i_new, zdst
            if t_zs is not None:
                nc.sync.dma_start(out=t_zs[:], in_=zs)

            if stage_limit < 12:
                return _finish_early(nc, out, pst, tap_handles, ctx)
            # ================= S12: LI readout =================
            # NOTE: LSNN is scaled x10 (tscr = 10*v_dec, z thresh 5.0) -- wait:
            # LSNN input xw is NOT scaled; z is 0/1 either way. See host notes.
            wt80_t = pbig.tile([P, T], F32, tag="SV")
            nc.sync.dma_start(out=wt80_t[:], in_=prm["wt80"][:])
            wt80s = wt80_t
            listrip = pst.tile([P, 16], F32, tag="listrip")
            dump = psc1.tile([P, 512], F32, tag="sc_s")
            for j in range(16):
                nc.vector.scalar_tensor_tensor(
                    out=dump[:], in0=zs[:, j * 512:(j + 1) * 512], scalar=1.0,
                    in1=wt80s[:, j * 512:(j + 1) * 512],
                    op0=OP.mult, op1=OP.mult, accum_out=listrip[:, j:j + 1])
            inner = pst.tile([P, 1], F32, tag="inner")
            nc.vector.tensor_reduce(out=inner[:], in_=listrip[:],
                                    axis=mybir.AxisListType.X, op=OP.add)
            if t_inner is not None:
                nc.sync.dma_start(out=t_inner[:], in_=inner[:])
            pso = pps2.tile([16, 1], F32, tag="small")
            nc.tensor.matmul(out=pso[:], lhsT=mclss[:], rhs=inner[:],
                             start=True, stop=True)
            outt = pst.tile([16, 1], F32, tag="outt")
            nc.vector.tensor_copy(out=outt[:], in_=pso[:])
            nc.sync.dma_start(out=out[:], in_=outt[:])

        if t_stats is not None:
            nc.sync.dma_start(out=t_stats[:], in_=coefs[:])
        ctx.close()
    return nc, tap_handles


# ---------------- host-side input prep ----------------
def li_weights():
    hv = np.zeros(T + 1)
    v_r, i_r = 0.0, 1.0
    for j in range(1, T + 1):
        v_r = 0.9 * v_r + 0.1 * i_r
        i_r = 0.8 * i_r
        hv[j] = v_r
    cum = np.cumsum(hv)
    w = np.array([cum[T - k] for k in range(1, T + 1)])
    return (w / T).astype(np.float32)


def blockdiag(w):
    """w [H,H] -> BD [P,P] with BD[(b,h1),(b,h2)] = w[h2,h1] (lhsT layout)."""
    bd = np.zeros((P, P), np.float32)
    for b in range(BL):
        bd[b * H:(b + 1) * H, b * H:(b + 1) * H] = w.T
    return bd


def host_inputs(full_inputs):
    """full_inputs: dict from setup_inputs(). Returns list of per-core in_maps."""
    f32 = lambda x: np.ascontiguousarray(np.asarray(x, np.float32))
    w_front = f32(full_inputs["w_front"])
    wf2 = np.concatenate([w_front.T, w_front.T], axis=0)  # [128, 10]
    common = {
        "wf2": wf2,
        "bd_q": blockdiag(2.0 * f32(full_inputs["wq"])),
        "bd_k": blockdiag(2.0 * f32(full_inputs["wk"])),
        "bd_v": blockdiag(2.0 * f32(full_inputs["wv"])),
        "bd_c": blockdiag(2.0 * f32(full_inputs["w_cma"])),
        "bd_in": blockdiag(2.0 * f32(full_inputs["w_in"])),
        "bd_rec": blockdiag(f32(full_inputs["w_rec"])),
    }
    ssel = np.zeros((P, P), np.float32)
    for b in range(BL):
        ssel[b * H:(b + 1) * H, b * H:(b + 1) * H] = 1.0
    common["s_sel"] = ssel
    common["i80f"] = np.eye(P, dtype=np.float32)
    ssum = np.zeros((P, H), np.float32)
    for b in range(BL):
        for h in range(H):
            ssum[b * H + h, h] = 1.0
    common["ssum"] = ssum
    r10 = np.zeros((H, P), np.float32)
    for b in range(BL):
        for h in range(H):
            r10[h, b * H + h] = 1.0
    common["r10"] = r10
    common["wt80"] = np.broadcast_to(li_weights()[None, :], (P, T)).copy()
    w_cls = f32(full_inputs["w_cls"])
    mcls = np.zeros((P, 16), np.float32)
    for b in range(BL):
        for h in range(H):
            for o in range(O):
                mcls[b * H + h, b * O + o] = w_cls[o, h]
    common["mcls"] = mcls
    gb = np.zeros((H, 10), np.float32)
    for i, (g, bta) in enumerate([("gi", "bi"), ("gq", "bq"), ("gk", "bk"),
                                  ("gv", "bv"), ("gc", "bc")]):
        gb[:, 2 * i] = f32(full_inputs[g])
        gb[:, 2 * i + 1] = f32(full_inputs[bta])
    common["gb"] = gb
    import ml_dtypes
    common["s_sel"] = common["s_sel"].astype(ml_dtypes.bfloat16)
    beeg = np.asarray(full_inputs["beeg"], np.float32)
    in_maps = []
    for c in range(N_CORES):
        m = dict(common)
        m["eeg"] = np.ascontiguousarray(beeg[c * BL:(c + 1) * BL, 0])
        in_maps.append(m)
    return in_maps


def assemble_out(results):
    """results: list of per-core {'out': [16,1]} -> [64, 2] f32."""
    outs = [r["out"].reshape(BL, O) for r in results]
    return np.concatenate(outs, axis=0).astype(np.float32)


# ---------------- public entry point ----------------
_CACHE = {}


def kernel(**inputs):
    """Full-input, full-output entry. Extra inputs (bs1, bs2, targets) ignored."""
    if "nc" not in _CACHE:
        _CACHE["nc"], _ = build_kernel(taps=())
    nc = _CACHE["nc"]
    in_maps = host_inputs(inputs)
    res = run_bass_kernel_spmd(nc, in_maps, list(range(N_CORES)))
    return assemble_out(res.results)



# revision 34
# speedup vs baseline: 6.4070x; 6.4070x over previous
"""Self-contained Trainium2 (Bass/Tile) kernel for the SNN problem — v2.

kernel(**inputs) takes FULL unsharded inputs (as from setup_inputs()) and
returns the FULL [64, 2] float32 output. Batch-sharded SPMD over 8 cores.

v2 changes vs baseline:
- all large matmuls run in bf16 (fp32 matmuls cost 2 PE passes each on
  TRN2; spike operands are exactly representable in bf16)
- LIF scans: membrane state written in-place into the input buffer
  (2 vector ops/step instead of 4 incl. slow gpsimd), spikes extracted in
  one bulk op afterwards (u == 0  <=>  spiked); warmup 12 -> 6 steps
- LSNN: 256 chunks x 32 steps, warmup 96 -> 64; all elementwise ops on
  the vector engine; z / xw matmuls in bf16
- banded attention: bf16 end-to-end (exact: spikes 0.5-valued, products
  multiples of 1/8), PSUM->SBUF moves on the scalar engine, shifted
  copies of K/V keep odd-diagonal bf16 operands 4-byte aligned
"""
import json
import numpy as np
from contextlib import ExitStack
import bass_rust
from concourse import bass, mybir, tile, bass2jax
from concourse.bass_utils import run_bass_kernel_spmd


# ---------- walrus workarounds (this container's compiler build) ----------
def _no_clear(self, sems):
    if not sems:
        return
    sem_nums = [s.num if hasattr(s, "num") else s for s in sems]
    self._state.prepend_free_semaphores(sem_nums)
    for ps in self._tile_sem_poison_stack:
        ps.update(sem_nums)


def _split_multi_waits(bir_json):
    d = json.loads(bir_json)
    changed = False
    for fn in d.get("functions", []):
        for blk in fn.get("blocks", []):
            insts = blk.get("instructions", [])
            out = []
            for inst in insts:
                si = inst.get("sync_info")
                if si:
                    ow = si.get("on_wait") or []
                    if len(ow) > 1:
                        changed = True
                        for j, w in enumerate(ow[:-1]):
                            out.append({
                                "debug": inst.get("debug", 0),
                                "engine": inst["engine"],
                                "ins": [], "outs": [],
                                "name": f"{inst['name']}-wsplit{j}",
                                "opcode": "NoOp",
                                "sync_info": {"on_update": [], "on_wait": [w]},
                            })
                        si["on_wait"] = [ow[-1]]
                out.append(inst)
            if len(out) != len(insts):
                blk["instructions"] = out
    return json.dumps(d).encode() if changed else bir_json


_orig_compile = bass2jax.compile_bir_kernel


def _patched_compile(bir_json, tmpdir, neff_name="file.neff"):
    return _orig_compile(_split_multi_waits(bir_json), tmpdir, neff_name=neff_name)


def _apply_patches():
    bass.Bass.clear_and_free_semaphores = _no_clear
    bass2jax.compile_bir_kernel = _patched_compile


_apply_patches()

DT = mybir.dt
F32 = DT.float32
F32R = DT.float32r
BF16 = DT.bfloat16
OP = mybir.AluOpType
AFT = mybir.ActivationFunctionType

N_CORES = 8
BL, C, T, H, O = 8, 64, 8192, 10, 2
P = BL * H  # 80
VTH = 0.5
TAU = 0.25
NCH, LC, WC = 512, 16, 8           # LIF chunks / len / warm
STEPS = LC + WC                     # 22
NCH2, LC2, WC2 = 256, 32, 96        # LSNN
STEPS2 = LC2 + WC2                  # 96
PADY = WC + T                       # 8198
HALO = 4
SPW = 2 * HALO + T                  # 8200
XWW = WC2 + T                       # 8256
EPS = 1e-5
NTOT = 64 * T
TH_AV = 0.625                       # S7 LIF threshold (av is 1.25x-scaled)
SVAL = 0.5                          # spike value stored in all spike bufs


def _finish_early(nc, out, pst, tap_handles, ctx):
    t = pst.tile([16, 1], F32, tag="outt")
    nc.vector.memset(t[:], 0.0)
    nc.sync.dma_start(out=out[:], in_=t[:])
    ctx.close()
    return nc, tap_handles


def build_kernel(taps=(), stage_limit=99):
    nc = bass.Bass()
    taps = set(taps)

    eeg = nc.declare_dram_parameter("eeg", [BL, C, T], F32, isOutput=False)
    wf2h = nc.declare_dram_parameter("wf2h", [128, 2 * H], BF16, isOutput=False)
    wf2l = nc.declare_dram_parameter("wf2l", [128, 2 * H], BF16, isOutput=False)
    prm = {}
    names = []
    for w in ("q", "k", "v", "c", "in", "rec"):
        names += [(f"bd_{w}h", [P, P], BF16), (f"bd_{w}l", [P, P], BF16)]
    for nm, shape, dt in names + [
            ("s_sel", [P, P], BF16), ("i80b", [P, P], BF16),
            ("i80r", [P, P], F32R), ("zpad", [P, WC2], F32R),
            ("ssum", [P, H], F32), ("r10", [H, P], F32),
            ("wt80", [P, T], F32), ("mcls", [P, 16], F32), ("gb", [H, 10], F32)]:
        prm[nm] = nc.declare_dram_parameter(nm, shape, dt, isOutput=False)

    out = nc.declare_dram_parameter("out", [16, 1], F32, isOutput=True)

    tap_handles = {}

    def tapdecl(name, shape, dtype):
        if name in taps:
            tap_handles[name] = nc.declare_dram_parameter(
                f"tap_{name}", shape, dtype, isOutput=True)
        return tap_handles.get(name)

    t_yf = tapdecl("yf", [P, PADY], F32)
    t_xspk = tapdecl("xspk", [P, T], BF16)
    t_q = tapdecl("qspk", [P, SPW], BF16)
    t_k = tapdecl("kspk", [P, SPW], BF16)
    t_v = tapdecl("vspk", [P, SPW], BF16)
    t_avx = tapdecl("avx", [P, PADY], F32)
    t_avspk = tapdecl("avspk", [P, T], BF16)
    t_ycma = tapdecl("ycma", [P, PADY], F32)
    t_av2 = tapdecl("av2spk", [P, T], BF16)
    t_xw = tapdecl("xw", [P, XWW], F32R)
    t_zs = tapdecl("zs", [P, T], BF16)
    t_stats = tapdecl("stats", [P, 16], F32)
    t_inner = tapdecl("inner", [P, 1], F32)
    t_qe = tapdecl("qspk_early", [P, SPW], BF16)
    t_qend = tapdecl("qspk_end", [P, SPW], BF16)
    t_xe = tapdecl("xspk_end", [P, T], BF16)
    t_qf32 = tapdecl("qspk_f32", [P, PADY], F32)
    t_listrip = tapdecl("listrip", [P, 16], F32)
    t_wtchk = tapdecl("wtchk", [P, T], F32)

    ctx = ExitStack()
    with tile.TileContext(nc) as tc:
        pp = ctx.enter_context(tc.tile_pool(name="params", bufs=1))
        pbig = ctx.enter_context(tc.tile_pool(name="big", bufs=1))
        pio = ctx.enter_context(tc.tile_pool(name="io", bufs=2))
        psc = ctx.enter_context(tc.tile_pool(name="scan", bufs=2))
        psc1 = ctx.enter_context(tc.tile_pool(name="scan1", bufs=2))
        pst = ctx.enter_context(tc.tile_pool(name="stats", bufs=1))
        pps1 = ctx.enter_context(tc.tile_pool(name="psum1", bufs=1, space="PSUM"))
        pps2 = ctx.enter_context(tc.tile_pool(name="psum2", bufs=2, space="PSUM"))
        pdram = ctx.enter_context(tc.tile_pool(name="dram", bufs=1, space="DRAM"))

        def ldparam(name):
            src = prm[name]
            t = pp.tile(list(src.shape), src.dtype, tag=f"prm_{name}")
            nc.sync.dma_start(out=t[:], in_=src[:])
            return t

        wf2hs = pp.tile([128, 2 * H], BF16, tag="prm_wf2h")
        nc.sync.dma_start(out=wf2hs[:], in_=wf2h[:])
        wf2ls = pp.tile([128, 2 * H], BF16, tag="prm_wf2l")
        nc.sync.dma_start(out=wf2ls[:], in_=wf2l[:])
        bdq = (ldparam("bd_qh"), ldparam("bd_ql"))
        bdk = (ldparam("bd_kh"), ldparam("bd_kl"))
        bdv = (ldparam("bd_vh"), ldparam("bd_vl"))
        bdc = (ldparam("bd_ch"), ldparam("bd_cl"))
        bdin = (ldparam("bd_inh"), ldparam("bd_inl"))
        bdrec = (ldparam("bd_rech"), ldparam("bd_recl"))
        ssel, identb = ldparam("s_sel"), ldparam("i80b")
        identr = ldparam("i80r")
        ssums, r10s = ldparam("ssum"), ldparam("r10")
        mclss, gbs = ldparam("mcls"), ldparam("gb")

        def mm_split(out_ap, bd2, rhs_ap):
            nc.tensor.matmul(out=out_ap, lhsT=bd2[0][:], rhs=rhs_ap,
                             start=True, stop=False)
            nc.tensor.matmul(out=out_ap, lhsT=bd2[1][:], rhs=rhs_ap,
                             start=False, stop=True)

        coefs = pst.tile([P, 16], F32, tag="coefs")

        # ---------------- BN stats -> global affine coefs ----------------
        def bn_to_global(ybufs, gcols, coffs, ar_name):
            nl = len(ybufs)
            K = 2 * nl
            strip = pst.tile([P, nl, 16, 6], F32, tag="bnstrip")
            for li, y in enumerate(ybufs):
                for j in range(16):
                    nc.vector.bn_stats(out=strip[:, li, j, :],
                                       in_=y[:, WC + j * 512: WC + (j + 1) * 512])
            ss = pst.tile([P, K], F32, tag="ss")
            mv = pst.tile([P, nl, 2], F32, tag="mv")
            for li in range(nl):
                nc.vector.bn_aggr(out=mv[:, li, :], in_=strip[:, li, :, :])
                mean = mv[:, li, 0:1]
                var = mv[:, li, 1:2]
                nc.vector.tensor_scalar(out=ss[:, 2 * li:2 * li + 1], in0=mean,
                                        scalar1=float(T), scalar2=None, op0=OP.mult)
                sq = ss[:, 2 * li + 1:2 * li + 2]
                nc.vector.scalar_tensor_tensor(out=sq, in0=mean, scalar=1.0,
                                               in1=mean, op0=OP.mult, op1=OP.mult)
                nc.vector.scalar_tensor_tensor(out=sq, in0=sq, scalar=1.0,
                                               in1=var, op0=OP.mult, op1=OP.add)
                nc.vector.tensor_scalar(out=sq, in0=sq, scalar1=float(T),
                                        scalar2=None, op0=OP.mult)
            pred = pps2.tile([H, K], F32, tag="small")
            nc.tensor.matmul(out=pred[:], lhsT=ssums[:], rhs=ss[:],
                             start=True, stop=True)
            loc = pst.tile([H, K], F32, tag="loc")
            nc.vector.tensor_copy(out=loc[:], in_=pred[:])
            ar_in = pdram.tile([H, K], F32, tag="arin")
            ar_out = pdram.tile([H, K], F32, tag="arout",
                                addr_space="Shared")
            nc.gpsimd.dma_start(out=ar_in[:], in_=loc[:])
            nc.gpsimd.collective_compute(
                "AllReduce", OP.add, replica_groups=[list(range(N_CORES))],
                ins=[ar_in.opt()], outs=[ar_out.opt()])
            glob = pst.tile([H, K], F32, tag="glob")
            nc.gpsimd.dma_start(out=glob[:], in_=ar_out[:])
            ac10 = pst.tile([H, K], F32, tag="ac10")
            tmp = pst.tile([H, 4], F32, tag="tmp")
            for li, gcol in enumerate(gcols):
                mean, var = tmp[:, 0:1], tmp[:, 1:2]
                nc.vector.tensor_scalar(out=mean, in0=glob[:, 2 * li:2 * li + 1],
                                        scalar1=1.0 / NTOT, scalar2=None, op0=OP.mult)
                nc.vector.tensor_scalar(out=var, in0=glob[:, 2 * li + 1:2 * li + 2],
                                        scalar1=1.0 / NTOT, scalar2=None, op0=OP.mult)
                nc.vector.scalar_tensor_tensor(out=tmp[:, 2:3], in0=mean, scalar=1.0,
                                               in1=mean, op0=OP.mult, op1=OP.mult)
                nc.vector.tensor_tensor(out=var, in0=var, in1=tmp[:, 2:3],
                                        op=OP.subtract)
                nc.vector.tensor_scalar(out=var, in0=var, scalar1=EPS,
                                        scalar2=None, op0=OP.add)
                nc.scalar.activation(out=var, in_=var, func=AFT.Sqrt)
                nc.vector.reciprocal(out=var, in_=var)
                a10 = ac10[:, 2 * li:2 * li + 1]
                nc.vector.scalar_tensor_tensor(out=a10, in0=var, scalar=VTH,
                                               in1=gbs[:, gcol:gcol + 1],
                                               op0=OP.mult, op1=OP.mult)
                nc.vector.scalar_tensor_tensor(out=tmp[:, 3:4], in0=a10, scalar=-1.0,
                                               in1=mean, op0=OP.mult, op1=OP.mult)
                nc.vector.tensor_tensor(out=ac10[:, 2 * li + 1:2 * li + 2],
                                        in0=gbs[:, gcol + 1:gcol + 2],
                                        in1=tmp[:, 3:4], op=OP.add)
            pbc = pps2.tile([P, K], F32, tag="small")
            nc.tensor.matmul(out=pbc[:], lhsT=r10s[:], rhs=ac10[:],
                             start=True, stop=True)
            for li, coff in enumerate(coffs):
                nc.vector.tensor_copy(out=coefs[:, coff:coff + 2],
                                      in_=pbc[:, 2 * li:2 * li + 2])

        def affine_inplace(ybuf, coff):
            a = coefs[:, coff:coff + 1]
            cst = coefs[:, coff + 1:coff + 2]
            for j in range(8):
                sl = ybuf[:, WC + j * 1024: WC + (j + 1) * 1024]
                nc.scalar.activation(out=sl, in_=sl, func=AFT.Identity,
                                     bias=cst, scale=a)

        # ---------------- chunk-parallel LIF scan, v2 ----------------
        # Warm steps keep the membrane u in ping-pong tiles; main steps
        # write u in-place over the input column just consumed (dead, and
        # its position class [s mod LC] is written exactly once since
        # s < WC+LC <= s'+LC for every other writer s'). Afterwards the
        # data region holds post-reset u for every timestep; a spike
        # happened exactly where u == 0 (P(scr==0) = 0 for continuous x).
        def lif_scan2(ybuf, spk_out, theta, sval, explicit_spikes=False):
            """spk_out: [P, T] region of a spike buffer.

            explicit_spikes=True writes spikes per main step (needed when the
            input can be exactly 0.0 for long stretches — e.g. raw attention
            sums — which would fool the bulk u==0 extraction)."""
            u_prev = None
            for s in range(STEPS):
                xcols = ybuf[:, s: s + (NCH - 1) * LC + 1: LC]
                if s == 0:
                    ut = psc1.tile([P, NCH], F32, tag="uwm")
                    nc.vector.scalar_tensor_tensor(
                        out=ut[:], in0=xcols, scalar=theta, in1=xcols,
                        op0=OP.is_lt, op1=OP.mult)
                    u_prev = ut[:]
                    continue
                scr = psc1.tile([P, NCH], F32, tag="scr")
                nc.vector.scalar_tensor_tensor(
                    out=scr[:], in0=u_prev, scalar=TAU, in1=xcols,
                    op0=OP.mult, op1=OP.add)
                if s >= WC:
                    udst = xcols
                    if explicit_spikes:
                        j = s - WC
                        spk_col = spk_out[:, j: j + (NCH - 1) * LC + 1: LC]
                        nc.vector.tensor_scalar(out=spk_col, in0=scr[:],
                                                scalar1=theta, scalar2=sval,
                                                op0=OP.is_ge, op1=OP.mult)
                else:
                    ut = psc1.tile([P, NCH], F32, tag="uwm")
                    udst = ut[:]
                nc.vector.scalar_tensor_tensor(
                    out=udst, in0=scr[:], scalar=theta, in1=scr[:],
                    op0=OP.is_lt, op1=OP.mult)
                u_prev = udst
            if not explicit_spikes:
                uall = ybuf[:, WC: WC + T]
                nc.vector.tensor_scalar(out=spk_out, in0=uall, scalar1=0.0,
                                        scalar2=sval, op0=OP.is_equal, op1=OP.mult)

        # ================= S1: front matmul (bf16) =================
        yb_t = pbig.tile([P, PADY], F32, tag="Y0")
        yf = yb_t[:]
        nc.vector.memset(yf[:, 0:WC], 0.0)
        for pair in range(4):
            b0, b1 = 2 * pair, 2 * pair + 1
            for quar in range(4):
                ps = pps1.tile([2 * H, 2048], F32, tag="big4")
                for i in range(2):
                    tl = pio.tile([128, 1024], F32, tag="eegtile")
                    ehi = pio.tile([128, 1024], BF16, tag="ehi")
                    elo = pio.tile([128, 1024], BF16, tag="elo")
                    base = quar * 2048 + i * 1024
                    nc.sync.dma_start(out=tl[0:64, :], in_=eeg[b0][:, base:base + 1024])
                    nc.sync.dma_start(out=tl[64:128, :], in_=eeg[b1][:, base:base + 1024])
                    nc.vector.tensor_copy(out=ehi[:], in_=tl[:])
                    nc.vector.tensor_tensor(out=elo[:], in0=tl[:], in1=ehi[:],
                                            op=OP.subtract)
                    for c5 in range(2):
                        col = i * 1024 + c5 * 512
                        sl = slice(c5 * 512, (c5 + 1) * 512)
                        nc.tensor.matmul(out=ps[:, col:col + 512], lhsT=wf2hs[:],
                                         rhs=ehi[:, sl], start=True, stop=False)
                        nc.tensor.matmul(out=ps[:, col:col + 512], lhsT=wf2hs[:],
                                         rhs=elo[:, sl], start=False, stop=False)
                        nc.tensor.matmul(out=ps[:, col:col + 512], lhsT=wf2ls[:],
                                         rhs=ehi[:, sl], start=False, stop=True)
                for hh in range(2):
                    stage = pio.tile([2 * H, 1024], F32, tag="stage")
                    nc.scalar.activation(out=stage[:], in_=ps[:, hh * 1024:(hh + 1) * 1024],
                                         func=AFT.Copy)
                    c0 = WC + quar * 2048 + hh * 1024
                    nc.sync.dma_start(out=yf[b0 * H:(b0 + 1) * H, c0:c0 + 1024],
                                      in_=stage[0:H, :])
                    nc.sync.dma_start(out=yf[b1 * H:(b1 + 1) * H, c0:c0 + 1024],
                                      in_=stage[H:2 * H, :])

        if stage_limit < 2:
            return _finish_early(nc, out, pst, tap_handles, ctx)
        # ================= S2: front BN =================
        bn_to_global([yf], [0], [0], "f")
        affine_inplace(yf, 0)
        if t_yf is not None:
            nc.sync.dma_start(out=t_yf[:], in_=yf)

        if stage_limit < 3:
            return _finish_early(nc, out, pst, tap_handles, ctx)
        # ================= S3: front LIF -> x spikes (bf16) =================
        tagA = pbig.tile([P, SPW], BF16, tag="A16")
        xspk = tagA[:, 0:T]
        lif_scan2(yf, xspk, VTH, SVAL)
        if t_xspk is not None:
            nc.sync.dma_start(out=t_xspk[:], in_=xspk)

        if stage_limit < 4:
            return _finish_early(nc, out, pst, tap_handles, ctx)
        # ================= S4/S5: q,k,v matmul + BN + LIF =================
        spkQ = pbig.tile([P, SPW], BF16, tag="SQ")
        spkK = pbig.tile([P, SPW], BF16, tag="SK")
        spkV = pbig.tile([P, SPW], BF16, tag="SV")
        for buf in (spkQ, spkK, spkV):
            nc.vector.memset(buf[:, 0:HALO], 0.0)
            nc.vector.memset(buf[:, HALO + T:], 0.0)
        for li, (bd, sbuf) in enumerate(((bdq, spkQ), (bdk, spkK), (bdv, spkV))):
            ysl_t = pbig.tile([P, PADY], F32, tag="Y0")
            ysl = ysl_t[:]
            nc.vector.memset(ysl[:, 0:WC], 0.0)
            for j in range(16):
                ps = pps2.tile([P, 512], F32, tag="small")
                mm_split(ps[:], bd, xspk[:, j * 512:(j + 1) * 512])
                nc.scalar.activation(out=ysl[:, WC + j * 512: WC + (j + 1) * 512],
                                     in_=ps[:], func=AFT.Copy)
            bn_to_global([ysl], [2 + 2 * li], [2 + 2 * li], "qkv")
            affine_inplace(ysl, 2 + 2 * li)
            lif_scan2(ysl, sbuf[:, HALO:HALO + T], VTH, SVAL)
            if li == 0 and t_qe is not None:
                nc.sync.dma_start(out=t_qe[:], in_=spkQ[:])
            if li == 0 and t_qf32 is not None:
                qstg = pbig.tile([P, PADY], F32, tag="Y0")
                nc.vector.tensor_copy(out=qstg[:], in_=spkQ[:, 0:PADY])
                nc.sync.dma_start(out=t_qf32[:], in_=qstg[:])
        if t_q is not None:
            nc.sync.dma_start(out=t_q[:], in_=spkQ[:])
            nc.sync.dma_start(out=t_k[:], in_=spkK[:])
            nc.sync.dma_start(out=t_v[:], in_=spkV[:])

        if stage_limit < 6:
            return _finish_early(nc, out, pst, tap_handles, ctx)
        # ================= S6: banded attention (bf16 exact) =================
        # shifted copies so odd-d operands stay 4B-aligned for 2x DVE mode
        spkK1_t = pbig.tile([P, SPW], BF16, tag="A16")
        spkV1_t = pbig.tile([P, SPW], BF16, tag="B16")
        spkK1, spkV1 = spkK1_t[:], spkV1_t[:]
        nc.scalar.activation(out=spkK1[:, 0:SPW - 1], in_=spkK[:, 1:SPW], func=AFT.Copy)
        nc.scalar.activation(out=spkV1[:, 0:SPW - 1], in_=spkV[:, 1:SPW], func=AFT.Copy)
        nc.vector.memset(spkK1[:, SPW - 1:], 0.0)
        nc.vector.memset(spkV1[:, SPW - 1:], 0.0)

        avx_t = pbig.tile([P, PADY], F32, tag="Y0")
        avx = avx_t[:]
        nc.vector.memset(avx[:, 0:WC], 0.0)
        for blk in range(8):
            psav = pps1.tile([P, 1024], F32, tag="med")
            for d in range(9):
                if d % 2 == 0:
                    k_ap = spkK[:, d + blk * 1024: d + blk * 1024 + 1024]
                    v_ap = spkV[:, d + blk * 1024: d + blk * 1024 + 1024]
                else:
                    k_ap = spkK1[:, (d - 1) + blk * 1024: (d - 1) + blk * 1024 + 1024]
                    v_ap = spkV1[:, (d - 1) + blk * 1024: (d - 1) + blk * 1024 + 1024]
                q_ap = spkQ[:, HALO + blk * 1024: HALO + (blk + 1) * 1024]
                prod = pio.tile([P, 1024], BF16, tag="prod")
                nc.vector.tensor_tensor(out=prod[:], in0=q_ap, in1=k_ap, op=OP.mult)
                pcnt_sb = pio.tile([P, 1024], BF16, tag="pcsb")
                for c2 in range(2):
                    pcnt = pps2.tile([P, 512], F32, tag="small")
                    nc.tensor.matmul(out=pcnt[:],
                                     lhsT=ssel[:],
                                     rhs=prod[:, c2 * 512:(c2 + 1) * 512],
                                     start=True, stop=True)
                    nc.scalar.activation(out=pcnt_sb[:, c2 * 512:(c2 + 1) * 512],
                                         in_=pcnt[:], func=AFT.Copy)
                rr = pio.tile([P, 1024], BF16, tag="rr2")
                nc.vector.tensor_tensor(out=rr[:], in0=pcnt_sb[:], in1=v_ap, op=OP.mult)
                for c2 in range(2):
                    nc.tensor.matmul(out=psav[:, c2 * 512:(c2 + 1) * 512],
                                     lhsT=identb[:],
                                     rhs=rr[:, c2 * 512:(c2 + 1) * 512],
                                     start=(d == 0), stop=(d == 8))
            nc.scalar.activation(out=avx[:, WC + blk * 1024: WC + (blk + 1) * 1024],
                                 in_=psav[:], func=AFT.Copy)
        if t_avx is not None:
            nc.sync.dma_start(out=t_avx[:], in_=avx)

        if stage_limit < 7:
            return _finish_early(nc, out, pst, tap_handles, ctx)
        # ================= S7: av LIF (av is 1.25x true => theta 0.625) =====
        avA = pbig.tile([P, SPW], BF16, tag="A16")
        avspk = avA[:, 0:T]
        lif_scan2(avx, avspk, TH_AV, SVAL, explicit_spikes=True)
        if t_avspk is not None:
            nc.sync.dma_start(out=t_avspk[:], in_=avspk)

        if stage_limit < 8:
            return _finish_early(nc, out, pst, tap_handles, ctx)
        # ================= S8: cma matmul + BN + LIF =================
        ycma_t = pbig.tile([P, PADY], F32, tag="Y0")
        ycma = ycma_t[:]
        nc.vector.memset(ycma[:, 0:WC], 0.0)
        for j in range(16):
            ps = pps2.tile([P, 512], F32, tag="small")
            mm_split(ps[:], bdc, avspk[:, j * 512:(j + 1) * 512])
            nc.scalar.activation(out=ycma[:, WC + j * 512: WC + (j + 1) * 512],
                                 in_=ps[:], func=AFT.Copy)
        bn_to_global([ycma], [8], [8], "c")
        affine_inplace(ycma, 8)
        if t_ycma is not None:
            nc.sync.dma_start(out=t_ycma[:], in_=ycma)
        avB = pbig.tile([P, SPW], BF16, tag="B16")
        av2spk = avB[:, 0:T]
        lif_scan2(ycma, av2spk, VTH, SVAL)
        if t_av2 is not None:
            nc.sync.dma_start(out=t_av2[:], in_=av2spk)

        if stage_limit < 10:
            return _finish_early(nc, out, pst, tap_handles, ctx)
        # ================= S10: xw = W_in @ av2spk (bf16) =================
        xw_t = pbig.tile([P, XWW], F32R, tag="XW")
        xw = xw_t[:]
        nc.sync.dma_start(out=xw[:, 0:WC2], in_=prm["zpad"][:])
        for j in range(16):
            ps = pps2.tile([P, 512], F32, tag="small")
            mm_split(ps[:], bdin, av2spk[:, j * 512:(j + 1) * 512])
            nc.scalar.activation(out=xw[:, WC2 + j * 512: WC2 + (j + 1) * 512],
                                 in_=ps[:], func=AFT.Copy)
        if t_xw is not None:
            nc.sync.dma_start(out=t_xw[:], in_=xw)

        if stage_limit < 11:
            return _finish_early(nc, out, pst, tap_handles, ctx)
        # ================= S11: LSNN scan =================
        zsA = pbig.tile([P, SPW], BF16, tag="A16")
        zs = zsA[:, 0:T]
        w_prev = psc.tile([P, NCH2], F32, tag="ls_w")
        i_prev = psc.tile([P, NCH2], F32, tag="ls_i")
        nc.vector.memset(w_prev[:], 0.0)
        nc.vector.memset(i_prev[:], 0.0)
        z_prev = None
        for s in range(STEPS2):
            tscr = psc1.tile([P, NCH2], F32, tag="ls_t")
            nc.vector.scalar_tensor_tensor(out=tscr[:], in0=w_prev[:], scalar=0.9,
                                           in1=i_prev[:], op0=OP.mult, op1=OP.add)
            if s >= WC2:
                j = s - WC2
                zdst = zs[:, j: j + (NCH2 - 1) * LC2 + 1: LC2]
            else:
                ztile = psc.tile([P, NCH2], BF16, tag="ls_z")
                zdst = ztile[:]
            nc.vector.tensor_scalar(out=zdst, in0=tscr[:], scalar1=10.0 * VTH,
                                    scalar2=None, op0=OP.is_ge)
            w_new = psc.tile([P, NCH2], F32, tag="ls_w")
            nc.vector.scalar_tensor_tensor(out=w_new[:], in0=tscr[:],
                                           scalar=10.0 * VTH, in1=tscr[:],
                                           op0=OP.is_lt, op1=OP.mult)
            psr = pps2.tile([P, NCH2], F32, tag="small")
            nc.tensor.matmul(out=psr[:], lhsT=identr[:],
                             rhs=xw[:, s: s + (NCH2 - 1) * LC2 + 1: LC2],
                             start=True, stop=(z_prev is None))
            if z_prev is not None:
                nc.tensor.matmul(out=psr[:], lhsT=bdrec[0][:], rhs=z_prev,
                                 start=False, stop=False)
                nc.tensor.matmul(out=psr[:], lhsT=bdrec[1][:], rhs=z_prev,
                                 start=False, stop=True)
            i_new = psc.tile([P, NCH2], F32, tag="ls_i")
            nc.vector.scalar_tensor_tensor(out=i_new[:], in0=i_prev[:], scalar=0.8,
                                           in1=psr[:], op0=OP.mult, op1=OP.add)
            w_prev, i_prev, z_prev = w_new, i_new, zdst
        if t_zs is not None:
            nc.sync.dma_start(out=t_zs[:], in_=zs)

        if stage_limit < 12:
            return _finish_early(nc, out, pst, tap_handles, ctx)
        # ================= S12: LI readout (closed form) =================
        wtB = pbig.tile([P, PADY], F32, tag="Y0")
        wt80s = wtB[:, 0:T]
        nc.sync.dma_start(out=wt80s, in_=prm["wt80"][:])
        listrip = pst.tile([P, 16], F32, tag="listrip")
        dump = psc1.tile([P, 512], F32, tag="dump")
        for j in range(16):
            nc.vector.scalar_tensor_tensor(
                out=dump[:], in0=zs[:, j * 512:(j + 1) * 512], scalar=1.0,
                in1=wt80s[:, j * 512:(j + 1) * 512],
                op0=OP.mult, op1=OP.mult, accum_out=listrip[:, j:j + 1])
        if t_listrip is not None:
            nc.sync.dma_start(out=t_listrip[:], in_=listrip[:])
        if t_wtchk is not None:
            nc.sync.dma_start(out=t_wtchk[:], in_=wt80s)
        inner = pst.tile([P, 1], F32, tag="inner")
        nc.vector.tensor_reduce(out=inner[:], in_=listrip[:],
                                axis=mybir.AxisListType.X, op=OP.add)
        if t_inner is not None:
            nc.sync.dma_start(out=t_inner[:], in_=inner[:])
        pso = pps2.tile([16, 1], F32, tag="small")
        nc.tensor.matmul(out=pso[:], lhsT=mclss[:], rhs=inner[:],
                         start=True, stop=True)
        outt = pst.tile([16, 1], F32, tag="outt")
        nc.vector.tensor_copy(out=outt[:], in_=pso[:])
        nc.sync.dma_start(out=out[:], in_=outt[:])

        if t_stats is not None:
            nc.sync.dma_start(out=t_stats[:], in_=coefs[:])
        if t_qend is not None:
            nc.sync.dma_start(out=t_qend[:], in_=spkQ[:])
        if t_xe is not None:
            nc.sync.dma_start(out=t_xe[:], in_=xspk)
        ctx.close()
    return nc, tap_handles


# ---------------- host-side input prep ----------------
def li_weights():
    hv = np.zeros(T + 1)
    v_r, i_r = 0.0, 1.0
    for j in range(1, T + 1):
        v_r = 0.9 * v_r + 0.1 * i_r
        i_r = 0.8 * i_r
        hv[j] = v_r
    cum = np.cumsum(hv)
    w = np.array([cum[T - k] for k in range(1, T + 1)])
    return (w / T).astype(np.float32)


def blockdiag(w):
    """w [H,H] -> BD [P,P] with BD[(b,h1),(b,h2)] = w[h2,h1] (lhsT layout)."""
    bd = np.zeros((P, P), np.float32)
    for b in range(BL):
        bd[b * H:(b + 1) * H, b * H:(b + 1) * H] = w.T
    return bd


def host_inputs(full_inputs):
    """full_inputs: dict from setup_inputs(). Returns list of per-core in_maps."""
    import ml_dtypes
    bf16 = ml_dtypes.bfloat16
    f32 = lambda x: np.ascontiguousarray(np.asarray(x, np.float32))
    w_front = f32(full_inputs["w_front"])
    wf2 = np.zeros((128, 2 * H), np.float32)  # block-diag: 2 batches stacked
    wf2[0:64, 0:H] = w_front.T
    wf2[64:128, H:2 * H] = w_front.T
    common = {}

    def split16(key, w):
        hi = w.astype(bf16)
        lo = (w - hi.astype(np.float32)).astype(bf16)
        common[key + "h"] = hi
        common[key + "l"] = lo

    split16("wf2", wf2)

    split16("bd_q", blockdiag(2.0 * f32(full_inputs["wq"])))
    split16("bd_k", blockdiag(2.0 * f32(full_inputs["wk"])))
    split16("bd_v", blockdiag(2.0 * f32(full_inputs["wv"])))
    split16("bd_c", blockdiag(2.0 * f32(full_inputs["w_cma"])))
    split16("bd_in", blockdiag(2.0 * f32(full_inputs["w_in"])))
    split16("bd_rec", blockdiag(f32(full_inputs["w_rec"])))
    ssel = np.zeros((P, P), np.float32)
    for b in range(BL):
        ssel[b * H:(b + 1) * H, b * H:(b + 1) * H] = 1.0
    common["s_sel"] = ssel.astype(bf16)
    common["i80b"] = np.eye(P, dtype=np.float32).astype(bf16)
    common["i80r"] = np.eye(P, dtype=np.float32)
    common["zpad"] = np.zeros((P, WC2), np.float32)
    ssum = np.zeros((P, H), np.float32)
    for b in range(BL):
        for h in range(H):
            ssum[b * H + h, h] = 1.0
    common["ssum"] = ssum
    r10 = np.zeros((H, P), np.float32)
    for b in range(BL):
        for h in range(H):
            r10[h, b * H + h] = 1.0
    common["r10"] = r10
    common["wt80"] = np.broadcast_to(
        li_weights()[None, :], (P, T)).copy()
    w_cls = f32(full_inputs["w_cls"])
    mcls = np.zeros((P, 16), np.float32)
    for b in range(BL):
        for h in range(H):
            for o in range(O):
                mcls[b * H + h, b * O + o] = w_cls[o, h]
    common["mcls"] = mcls
    gb = np.zeros((H, 10), np.float32)
    for i, (g, bta) in enumerate([("gi", "bi"), ("gq", "bq"), ("gk", "bk"),
                                  ("gv", "bv"), ("gc", "bc")]):
        gb[:, 2 * i] = f32(full_inputs[g])
        gb[:, 2 * i + 1] = f32(full_inputs[bta])
    common["gb"] = gb
    beeg = np.asarray(full_inputs["beeg"], np.float32)
    in_maps = []
    for c in range(N_CORES):
        m = dict(common)
        m["eeg"] = np.ascontiguousarray(beeg[c * BL:(c + 1) * BL, 0])
        in_maps.append(m)
    return in_maps


def assemble_out(results):
    """results: list of per-core {'out': [16,1]} -> [64, 2] f32."""
    outs = [r["out"].reshape(BL, O) for r in results]
    return np.concatenate(outs, axis=0).astype(np.float32)


# ---------------- public entry point ----------------
_CACHE = {}


def kernel(**inputs):
    """Full-input, full-output entry. Extra inputs (bs1, bs2, targets) ignored."""
    if "nc" not in _CACHE:
        _CACHE["nc"], _ = build_kernel(taps=())
    nc = _CACHE["nc"]
    in_maps = host_inputs(inputs)
    res = run_bass_kernel_spmd(nc, in_maps, list(range(N_CORES)))
    return assemble_out(res.results)
